# revision 22
# baseline (speedup 1.0000x reference)
"""Inverse-DWT step (zero-upsample by 2 + circular conv with time-reversed
8-tap filters + sum) on 8 Trainium2 NeuronCores.

Math: the reference's ortho-normalized FFT conv reduces exactly to a 4-tap
polyphase circular convolution per output parity:

  out[r, 2p]   = sum_t ( w[2t+1]*d[r, p-3+t] + s[2t+1]*a[r, p-3+t] ) / 128
  out[r, 2p+1] = sum_t ( w[2t]  *d[r, p-3+t] + s[2t]  *a[r, p-3+t] ) / 128

(indices mod m, t in 0..3, 128 = sqrt(2*m) ortho scale).

Sharding: pure data parallelism over rows (2048 rows / 8 cores = 256 each),
filters replicated. Host-side prep packs d and a (with a 3-column circular
halo prepended) into one tensor so each on-device tile needs a single DMA.

Constraint honored throughout: each engine instruction can carry at most ONE
sync wait (ISA EVENTS struct has a single wait slot; walrus rejects more), so
dependencies are funneled through same-engine chains.
"""

import numpy as np
from contextlib import ExitStack

import concourse.bass as bass
import concourse.bacc as bacc
import concourse.mybir as mybir
import concourse.tile as tile
from concourse.bass_utils import run_bass_kernel_spmd

N_CORES = 8
N_ROWS, M = 2048, 8192
R = N_ROWS // N_CORES  # 256 rows per core
P = 128                # partitions
TAPS = 8
HALO = 3
ME = M + HALO          # extended row length
F32 = mybir.dt.float32

_NC_CACHE = {}


def _build_v1():
    """DVE polyphase kernel: 16 FMA passes per output chunk.

    Inputs (per core):
      da: [R, 2, M+3] f32 -- row r holds [d_ext; a_ext], each with 3-col
          circular halo prepended (d_ext[:, 0:3] = d[:, M-3:M]).
      ws: [16] f32 -- wavelet taps then scaling taps.
    Output: out [R, 2*M] f32.
    """
    nc = bacc.Bacc()
    da_ext = nc.declare_dram_parameter("da", [R, 2, ME], F32, isOutput=False)
    ws_ext = nc.declare_dram_parameter("ws", [2 * TAPS], F32, isOutput=False)
    out_ext = nc.declare_dram_parameter("out", [R, 2 * M], F32, isOutput=True)

    CHUNK = 2048  # input pairs per output chunk
    n_blocks = R // P          # 2
    n_chunks = M // CHUNK      # 4
    inv_scale = 1.0 / 128.0

    with tile.TileContext(nc) as tc, ExitStack() as ctx:
        const_pool = ctx.enter_context(tc.tile_pool(name="const", bufs=1))
        in_pool = ctx.enter_context(tc.tile_pool(name="inp", bufs=3))
        out_pool = ctx.enter_context(tc.tile_pool(name="outp", bufs=3))
        psum_pool = ctx.enter_context(tc.tile_pool(name="ps", bufs=1, space="PSUM"))

        # Broadcast the 16 filter taps (scaled by 1/128) to all 128
        # partitions via an outer-product matmul against a 1/128-valued ones
        # row. Operand producers are funneled through DVE so the fp32 matmul
        # (whose weight load rides the same instruction) needs one wait.
        wv_stage = const_pool.tile([1, 2 * TAPS], F32)
        nc.sync.dma_start(wv_stage[0:1, :],
                          ws_ext[:].rearrange("(o t) -> o t", o=1))
        wv = const_pool.tile([1, 2 * TAPS], F32)
        nc.vector.tensor_copy(wv[:], wv_stage[:])
        ones = const_pool.tile([1, P], F32)
        nc.vector.memset(ones[:], inv_scale)
        taps_ps = psum_pool.tile([P, 2 * TAPS], F32)
        nc.tensor.matmul(taps_ps[:], ones[:], wv[:], start=True, stop=True)
        taps_b = const_pool.tile([P, 2 * TAPS], F32)
        nc.vector.tensor_copy(taps_b[:], taps_ps[:])

        touch = const_pool.tile([P, 1], F32)

        for b in range(n_blocks):
            rows = slice(b * P, (b + 1) * P)
            for c in range(n_chunks):
                # one DMA brings this chunk's d slice and a slice (with halo)
                da_t = in_pool.tile([P, 2, CHUNK + HALO], F32)
                nc.sync.dma_start(
                    da_t[:], da_ext[rows, :, CHUNK * c: CHUNK * (c + 1) + HALO])
                # absorb the DMA completion wait into DVE's program order so
                # the first FMA below carries only the out-tile WAR wait
                nc.vector.tensor_copy(touch[:], da_t[:, 0, 0:1])

                o_t = out_pool.tile([P, 2 * CHUNK], F32)
                for par in (0, 1):
                    view = o_t[:, par::2]
                    for src_i in (0, 1):  # 0: d with w taps, 1: a with s taps
                        for t in range(4):
                            src = da_t[:, src_i, t: t + CHUNK]
                            ti = src_i * TAPS + 2 * t + 1 - par
                            sc = taps_b[:, ti: ti + 1]
                            if src_i == 0 and t == 0:
                                nc.vector.tensor_scalar_mul(view, src, sc)
                            else:
                                nc.vector.scalar_tensor_tensor(
                                    view, src, sc, view,
                                    op0=mybir.AluOpType.mult,
                                    op1=mybir.AluOpType.add)
                nc.sync.dma_start(
                    out_ext[rows, 2 * CHUNK * c: 2 * CHUNK * (c + 1)], o_t[:])
    nc.finalize()
    return nc


def _build_v2(mm_dtype="bf16"):
    """TensorEngine banded-matmul kernel.

    Per 128-row block: PE-transpose each 128-col block of d and a into
    dT/aT (partitions = input cols), then each 128-pair output block j is
    4 accumulating matmuls in PSUM:
        out[:, 256j:256j+256] = dT_j.T @ Wd + aT_j.T @ Ws
                              + dT_{j-1}.T @ Wd_halo + aT_{j-1}.T @ Ws_halo
    Wd/Ws are [128, 256] banded (4 taps per column); the halo matrices are
    [128, 6], zero except rows 125..127 — they add the 3-column circular
    carry from the previous block (j-1 wraps to 63).

    mm_dtype: "bf16" (inputs cast during DMA), "f32r" (fp32 storage,
    full-rate PE mode), or "f32" (exact, 4 cyc/row matmuls).
    """
    nc = bacc.Bacc()
    da_ext = nc.declare_dram_parameter("da", [R, 2, ME], F32, isOutput=False)
    ws_ext = nc.declare_dram_parameter("ws", [2 * TAPS], F32, isOutput=False)
    out_ext = nc.declare_dram_parameter("out", [R, 2 * M], F32, isOutput=True)

    BF16 = mybir.dt.bfloat16
    in_dt = BF16 if mm_dtype == "bf16" else F32
    mm_dt = {"bf16": BF16, "f32r": mybir.dt.float32r, "f32": F32}[mm_dtype]
    n_blocks = R // P             # 2 row-blocks
    NJ = M // P                   # 64 col-blocks per row-block
    CHUNK = 2048                  # input cols per load chunk (16 col-blocks)
    n_chunks = M // CHUNK         # 4
    JPC = CHUNK // P              # 16 col-blocks per chunk
    WCOLS = 2 * P + 6             # 262: main 256 + halo 6
    inv_scale = 1.0 / 128.0

    with tile.TileContext(nc) as tc, ExitStack() as ctx:
        const_pool = ctx.enter_context(tc.tile_pool(name="const", bufs=1))
        in_pool = ctx.enter_context(tc.tile_pool(name="inp", bufs=3))
        tr_pool = ctx.enter_context(
            tc.tile_pool(name="tr", bufs=2 if in_dt is BF16 else 1))
        out_pool = ctx.enter_context(tc.tile_pool(name="outp", bufs=3))
        ps_t_pool = ctx.enter_context(tc.tile_pool(name="pst", bufs=3, space="PSUM"))
        ps_o_pool = ctx.enter_context(tc.tile_pool(name="pso", bufs=3, space="PSUM"))
        ps_c_pool = ctx.enter_context(tc.tile_pool(name="psc", bufs=1, space="PSUM"))

        # ---- taps broadcast to all partitions (scaled by 1/128) ----
        wv_stage = const_pool.tile([1, 2 * TAPS], F32)
        nc.sync.dma_start(wv_stage[0:1, :],
                          ws_ext[:].rearrange("(o t) -> o t", o=1))
        wv = const_pool.tile([1, 2 * TAPS], F32)
        nc.vector.tensor_copy(wv[:], wv_stage[:])
        ones = const_pool.tile([1, P], F32)
        nc.vector.memset(ones[:], inv_scale)
        taps_ps = ps_c_pool.tile([P, 2 * TAPS], F32)
        nc.tensor.matmul(taps_ps[:], ones[:], wv[:], start=True, stop=True)
        taps_b = const_pool.tile([P, 2 * TAPS], F32)
        nc.vector.tensor_copy(taps_b[:], taps_ps[:])

        # ---- identity for PE transposes ----
        ident_f = const_pool.tile([P, P], F32)
        nc.gpsimd.memset(ident_f[:], 0.0)
        nc.gpsimd.affine_select(
            out=ident_f[:], in_=ident_f[:],
            compare_op=mybir.AluOpType.not_equal, fill=1.0,
            base=0, pattern=[[-1, P]], channel_multiplier=1)
        if in_dt is BF16:
            ident = const_pool.tile([P, P], BF16)
            nc.vector.tensor_copy(ident[:], ident_f[:])
        else:
            ident = ident_f

        # ---- banded weight matrices built from runtime taps ----
        # Wf[c, 2q+par]       = tap[2(c-q+3)+1-par] / 128   for 0 <= c-q+3 < 4
        # Wf[c, 256+2q+par]   = tap[2(c-125-q)+1-par] / 128 for c in 125..127
        wd_f = const_pool.tile([P, WCOLS], F32)
        ws_f = const_pool.tile([P, WCOLS], F32)
        nc.vector.memset(wd_f[:], 0.0)
        nc.vector.memset(ws_f[:], 0.0)
        mask = const_pool.tile([P, WCOLS], F32)
        for t in range(4):
            for par in (0, 1):
                nc.gpsimd.memset(mask[:], 0.0)
                # main band: zero-line at n - 2c - (6 - 2t + par) == 0
                nc.gpsimd.affine_select(
                    out=mask[:, 0:2 * P], in_=mask[:, 0:2 * P],
                    compare_op=mybir.AluOpType.not_equal, fill=1.0,
                    base=-(6 - 2 * t + par), pattern=[[1, 2 * P]],
                    channel_multiplier=-2)
                # halo band: zero-line at n_loc - 2c + 250 + 2t + par == 0
                nc.gpsimd.affine_select(
                    out=mask[:, 2 * P:WCOLS], in_=mask[:, 2 * P:WCOLS],
                    compare_op=mybir.AluOpType.not_equal, fill=1.0,
                    base=250 + 2 * t - par, pattern=[[1, 6]],
                    channel_multiplier=-2)
                ti = 2 * t + 1 - par
                nc.vector.scalar_tensor_tensor(
                    wd_f[:], mask[:], taps_b[:, ti:ti + 1], wd_f[:],
                    op0=mybir.AluOpType.mult, op1=mybir.AluOpType.add)
                nc.vector.scalar_tensor_tensor(
                    ws_f[:], mask[:], taps_b[:, TAPS + ti:TAPS + ti + 1], ws_f[:],
                    op0=mybir.AluOpType.mult, op1=mybir.AluOpType.add)
        if mm_dt is F32:
            wd, ws_t = wd_f, ws_f
        else:
            wd = const_pool.tile([P, WCOLS], mm_dt)
            ws_t = const_pool.tile([P, WCOLS], mm_dt)
            nc.vector.tensor_copy(wd[:], wd_f[:])
            nc.vector.tensor_copy(ws_t[:], ws_f[:])

        def copy_act(dst, srcp):
            return nc.scalar.copy(dst, srcp)

        def copy_dve(dst, srcp):
            return nc.vector.tensor_copy(dst, srcp)

        copy_engines = [copy_act, copy_dve]

        for b in range(n_blocks):
            rows = slice(b * P, (b + 1) * P)
            dT = tr_pool.tile([P, NJ * P], mm_dt, tag="dT")
            aT = tr_pool.tile([P, NJ * P], mm_dt, tag="aT")

            # ---- load + transpose ----
            ci = 0
            for c in range(n_chunks):
                da_t = in_pool.tile([P, 2, CHUNK], in_dt)
                src = da_ext[rows, :, HALO + CHUNK * c: HALO + CHUNK * (c + 1)]
                if in_dt is BF16:
                    nc.gpsimd.dma_start(da_t[:], src)   # SWDGE cast f32->bf16
                else:
                    nc.sync.dma_start(da_t[:], src)
                # 4 transposes per PSUM bank, one batched copy out
                for grp in range(JPC * 2 // 4):  # 8 groups of 4 blocks
                    ps = ps_t_pool.tile([P, 4 * P], in_dt)
                    for k in range(4):
                        idx = grp * 4 + k        # 0..31 within chunk
                        src_i, jj = idx % 2, idx // 2
                        nc.tensor.matmul(
                            ps[:, k * P:(k + 1) * P],
                            da_t[:, src_i, jj * P:(jj + 1) * P],
                            ident[:], is_transpose=True,
                            start=(k == 0), stop=(k == 3))
                    # copy the 4 transposed blocks to their dT/aT slots
                    for k in range(4):
                        idx = grp * 4 + k
                        src_i, jj = idx % 2, idx // 2
                        j = c * JPC + jj
                        dst = dT if src_i == 0 else aT
                        eng = copy_engines[ci % 2]; ci += 1
                        eng(dst[:, j * P:(j + 1) * P],
                            ps[:, k * P:(k + 1) * P])

            # ---- conv matmuls: 2 output blocks (512 cols) per PSUM bank ----
            ci = 0
            groups = list(range(1, NJ // 2)) + [0]  # group 0 last (j=0 wraps)
            stage = None
            for gi, g in enumerate(groups):
                ps = ps_o_pool.tile([P, 2 * 2 * P], F32)
                first = True
                for jj in (2 * g, 2 * g + 1):
                    jp = (jj - 1) % NJ
                    off = (jj - 2 * g) * 2 * P
                    for srcT, wmat in ((dT, wd), (aT, ws_t)):
                        nc.tensor.matmul(
                            ps[:, off:off + 2 * P],
                            srcT[:, jj * P:(jj + 1) * P],
                            wmat[:, 0:2 * P],
                            start=first, stop=False)
                        first = False
                        nc.tensor.matmul(
                            ps[:, off:off + 6],
                            srcT[:, jp * P:(jp + 1) * P],
                            wmat[:, 2 * P:WCOLS],
                            start=False,
                            stop=(jj == 2 * g + 1 and srcT is aT))
                # stage 2 bank-groups (512 cols each) per tile, then DMA
                if stage is None:
                    stage = out_pool.tile([P, 1024], F32)
                    stage_groups = []
                half = len(stage_groups)
                eng = copy_engines[ci % 2]; ci += 1
                eng(stage[:, half * 512:(half + 1) * 512], ps[:])
                stage_groups.append(g)
                if len(stage_groups) == 2:
                    g0, g1 = stage_groups
                    if g1 == g0 + 1:
                        nc.sync.dma_start(
                            out_ext[rows, 512 * g0: 512 * g0 + 1024],
                            stage[:])
                    else:
                        nc.sync.dma_start(
                            out_ext[rows, 512 * g0: 512 * (g0 + 1)],
                            stage[:, 0:512])
                        nc.sync.dma_start(
                            out_ext[rows, 512 * g1: 512 * (g1 + 1)],
                            stage[:, 512:1024])
                    stage = None
            if stage is not None:
                g0 = stage_groups[0]
                nc.sync.dma_start(
                    out_ext[rows, 512 * g0: 512 * (g0 + 1)],
                    stage[:, 0:512])
    nc.finalize()
    return nc


def _build_v3(mm_dtype="bf16"):
    """TensorEngine kernel with overlapped transpose windows (no halo mms).

    Window j (stride Q=125 pairs) transposes ext cols [125j, 125j+128) --
    which includes the 3-col halo -- so each 125-pair output block needs just
    ONE matmul per input: out[:, 250j:250j+250] = dT_j.T @ W (K=128, N=250).
    66 windows per 128-row block; the last covers 67 pairs (K=70, N=134).
    """
    nc = bacc.Bacc()
    da_ext = nc.declare_dram_parameter("da", [R, 2, ME], F32, isOutput=False)
    ws_ext = nc.declare_dram_parameter("ws", [2 * TAPS], F32, isOutput=False)
    out_ext = nc.declare_dram_parameter("out", [R, 2 * M], F32, isOutput=True)

    BF16 = mybir.dt.bfloat16
    in_dt = BF16 if mm_dtype == "bf16" else F32
    mm_dt = {"bf16": BF16, "f32r": mybir.dt.float32r, "f32": F32}[mm_dtype]
    n_blocks = R // P          # 2 row-blocks
    Q = 125                    # pairs per window
    NW = -(-M // Q)            # 66 windows per row-block
    NQUAD = -(-NW // 4)        # 17 transpose quads
    NG = NW // 2               # 33 conv groups (2 windows each)
    inv_scale = 1.0 / 128.0
    bufs_big = 2 if in_dt is BF16 else 1

    def win_geom(j):
        q0 = Q * j
        npairs = min(Q, M - q0)
        return q0, npairs, npairs + 3   # out-pair offset, n pairs, K

    with tile.TileContext(nc) as tc, ExitStack() as ctx:
        const_pool = ctx.enter_context(tc.tile_pool(name="const", bufs=1))
        in_pool = ctx.enter_context(tc.tile_pool(name="inp", bufs=bufs_big))
        tr_pool = ctx.enter_context(tc.tile_pool(name="tr", bufs=bufs_big))
        out_pool = ctx.enter_context(tc.tile_pool(name="outp", bufs=4))
        ps_t_pool = ctx.enter_context(tc.tile_pool(name="pst", bufs=3, space="PSUM"))
        ps_o_pool = ctx.enter_context(tc.tile_pool(name="pso", bufs=3, space="PSUM"))
        ps_c_pool = ctx.enter_context(tc.tile_pool(name="psc", bufs=1, space="PSUM"))

        # ---- taps broadcast (scaled by 1/128) ----
        wv_stage = const_pool.tile([1, 2 * TAPS], F32)
        nc.sync.dma_start(wv_stage[0:1, :],
                          ws_ext[:].rearrange("(o t) -> o t", o=1))
        wv = const_pool.tile([1, 2 * TAPS], F32)
        nc.vector.tensor_copy(wv[:], wv_stage[:])
        ones = const_pool.tile([1, P], F32)
        nc.vector.memset(ones[:], inv_scale)
        taps_ps = ps_c_pool.tile([P, 2 * TAPS], F32)
        nc.tensor.matmul(taps_ps[:], ones[:], wv[:], start=True, stop=True)
        taps_b = const_pool.tile([P, 2 * TAPS], F32)
        nc.vector.tensor_copy(taps_b[:], taps_ps[:])

        # ---- identity for PE transposes ----
        ident_f = const_pool.tile([P, P], F32)
        nc.gpsimd.memset(ident_f[:], 0.0)
        nc.gpsimd.affine_select(
            out=ident_f[:], in_=ident_f[:],
            compare_op=mybir.AluOpType.not_equal, fill=1.0,
            base=0, pattern=[[-1, P]], channel_multiplier=1)
        if in_dt is F32:
            ident = ident_f
        else:
            ident = const_pool.tile([P, P], in_dt)
            nc.vector.tensor_copy(ident[:], ident_f[:])

        # ---- banded weights: W[c, 2q+par] = tap[2(c-q)+1-par], 0<=c-q<4 ----
        wd_f = const_pool.tile([P, 2 * Q], F32)
        ws_f = const_pool.tile([P, 2 * Q], F32)
        nc.vector.memset(wd_f[:], 0.0)
        nc.vector.memset(ws_f[:], 0.0)
        mask = const_pool.tile([P, 2 * Q], F32)
        for t in range(4):
            for par in (0, 1):
                nc.gpsimd.memset(mask[:], 0.0)
                # zero-line: n - 2c + 2t - par == 0
                nc.gpsimd.affine_select(
                    out=mask[:], in_=mask[:],
                    compare_op=mybir.AluOpType.not_equal, fill=1.0,
                    base=2 * t - par, pattern=[[1, 2 * Q]],
                    channel_multiplier=-2)
                ti = 2 * t + 1 - par
                nc.vector.scalar_tensor_tensor(
                    wd_f[:], mask[:], taps_b[:, ti:ti + 1], wd_f[:],
                    op0=mybir.AluOpType.mult, op1=mybir.AluOpType.add)
                nc.vector.scalar_tensor_tensor(
                    ws_f[:], mask[:], taps_b[:, TAPS + ti:TAPS + ti + 1], ws_f[:],
                    op0=mybir.AluOpType.mult, op1=mybir.AluOpType.add)
        if mm_dt is F32:
            wd, ws_t = wd_f, ws_f
        else:
            wd = const_pool.tile([P, 2 * Q], mm_dt)
            ws_t = const_pool.tile([P, 2 * Q], mm_dt)
            nc.vector.tensor_copy(wd[:], wd_f[:])
            nc.vector.tensor_copy(ws_t[:], ws_f[:])

        def copy_act(dst, srcp):
            return nc.scalar.copy(dst, srcp)

        def copy_dve(dst, srcp):
            return nc.vector.tensor_copy(dst, srcp)

        copy_engines = [copy_act, copy_dve]

        for b in range(n_blocks):
            rows = slice(b * P, (b + 1) * P)
            da_t = in_pool.tile([P, 2, ME], in_dt)
            src = da_ext[rows, :, :]
            if in_dt is BF16:
                nc.gpsimd.dma_start(da_t[:], src)   # SWDGE cast f32->bf16
            else:
                nc.sync.dma_start(da_t[:], src)

            dT = tr_pool.tile([P, NW * P], mm_dt, tag="dT")
            aT = tr_pool.tile([P, NW * P], mm_dt, tag="aT")

            stage = None
            for qd in range(NQUAD):
                wins = [j for j in range(4 * qd, min(4 * qd + 4, NW))]
                # transpose quad for each input, one batched copy out each
                for src_i, dst in ((0, dT), (1, aT)):
                    ps = ps_t_pool.tile([P, 4 * P], in_dt)
                    for k, j in enumerate(wins):
                        _, npairs, K = win_geom(j)
                        nc.tensor.matmul(
                            ps[0:K, k * P:(k + 1) * P],
                            da_t[:, src_i, Q * j: Q * j + K],
                            ident[:], is_transpose=True,
                            start=(k == 0), stop=(k == len(wins) - 1))
                    eng = copy_engines[(qd + src_i) % 2]
                    eng(dst[:, 4 * qd * P: (4 * qd + len(wins)) * P],
                        ps[:, 0: len(wins) * P])
                # conv groups whose windows live in this quad
                for g in (2 * qd, 2 * qd + 1):
                    if g >= NG:
                        continue
                    ps = ps_o_pool.tile([P, 4 * Q], F32)
                    first = True
                    off = 0
                    widths = []
                    for j in (2 * g, 2 * g + 1):
                        _, npairs, K = win_geom(j)
                        N = 2 * npairs
                        for srcT, wmat in ((dT, wd), (aT, ws_t)):
                            nc.tensor.matmul(
                                ps[:, off:off + N],
                                srcT[0:K, j * P:(j + 1) * P],
                                wmat[0:K, 0:N],
                                start=first,
                                stop=(j == 2 * g + 1 and srcT is aT))
                            first = False
                        widths.append(N)
                        off += N
                    gw = sum(widths)   # 500, or 384 for the last group
                    if stage is None:
                        stage = out_pool.tile([P, 4 * Q], F32)
                        stage_off = 0
                        stage_out0 = 500 * g
                    eng = copy_engines[g % 2]
                    eng(stage[:, stage_off:stage_off + gw], ps[:, 0:gw])
                    stage_off += gw
                    if stage_off >= 4 * Q or g == NG - 1:
                        nc.sync.dma_start(
                            out_ext[rows, stage_out0: stage_out0 + stage_off],
                            stage[:, 0:stage_off])
                        stage = None
    nc.finalize()
    return nc


def _build_v4(mm_dtype="bf16"):
    """TensorEngine kernel with overlapped transpose windows (no halo mms).

    As v3, plus: loads are chunked (4 quad-aligned chunks per row-block)
    for fine-grained DMA/compute overlap, and stores are batched into
    [128, 4000] stages issued alternately on the two HWDGE queues.
    """
    nc = bacc.Bacc()
    da_ext = nc.declare_dram_parameter("da", [R, 2, ME], F32, isOutput=False)
    ws_ext = nc.declare_dram_parameter("ws", [2 * TAPS], F32, isOutput=False)
    out_ext = nc.declare_dram_parameter("out", [R, 2 * M], F32, isOutput=True)

    BF16 = mybir.dt.bfloat16
    in_dt = BF16 if mm_dtype == "bf16" else F32
    mm_dt = {"bf16": BF16, "f32r": mybir.dt.float32r, "f32": F32}[mm_dtype]
    n_blocks = R // P          # 2 row-blocks
    Q = 125                    # pairs per window
    NW = -(-M // Q)            # 66 windows per row-block
    NQUAD = -(-NW // 4)        # 17 transpose quads
    NG = NW // 2               # 33 conv groups (2 windows each)
    inv_scale = 1.0 / 128.0
    bufs_big = 2 if in_dt is BF16 else 1

    def win_geom(j):
        q0 = Q * j
        npairs = min(Q, M - q0)
        return q0, npairs, npairs + 3   # out-pair offset, n pairs, K

    with tile.TileContext(nc) as tc, ExitStack() as ctx:
        const_pool = ctx.enter_context(tc.tile_pool(name="const", bufs=1))
        in_pool = ctx.enter_context(tc.tile_pool(name="inp", bufs=3))
        tr_pool = ctx.enter_context(tc.tile_pool(name="tr", bufs=bufs_big))
        out_pool = ctx.enter_context(tc.tile_pool(name="outp", bufs=4))
        ps_t_pool = ctx.enter_context(tc.tile_pool(name="pst", bufs=3, space="PSUM"))
        ps_o_pool = ctx.enter_context(tc.tile_pool(name="pso", bufs=3, space="PSUM"))
        ps_c_pool = ctx.enter_context(tc.tile_pool(name="psc", bufs=1, space="PSUM"))

        # ---- taps broadcast (scaled by 1/128) ----
        wv_stage = const_pool.tile([1, 2 * TAPS], F32)
        nc.sync.dma_start(wv_stage[0:1, :],
                          ws_ext[:].rearrange("(o t) -> o t", o=1))
        wv = const_pool.tile([1, 2 * TAPS], F32)
        nc.vector.tensor_copy(wv[:], wv_stage[:])
        ones = const_pool.tile([1, P], F32)
        nc.vector.memset(ones[:], inv_scale)
        taps_ps = ps_c_pool.tile([P, 2 * TAPS], F32)
        nc.tensor.matmul(taps_ps[:], ones[:], wv[:], start=True, stop=True)
        taps_b = const_pool.tile([P, 2 * TAPS], F32)
        nc.vector.tensor_copy(taps_b[:], taps_ps[:])

        # ---- identity for PE transposes ----
        ident_f = const_pool.tile([P, P], F32)
        nc.gpsimd.memset(ident_f[:], 0.0)
        nc.gpsimd.affine_select(
            out=ident_f[:], in_=ident_f[:],
            compare_op=mybir.AluOpType.not_equal, fill=1.0,
            base=0, pattern=[[-1, P]], channel_multiplier=1)
        if in_dt is F32:
            ident = ident_f
        else:
            ident = const_pool.tile([P, P], in_dt)
            nc.vector.tensor_copy(ident[:], ident_f[:])

        # ---- banded weights: W[c, 2q+par] = tap[2(c-q)+1-par], 0<=c-q<4 ----
        wd_f = const_pool.tile([P, 2 * Q], F32)
        ws_f = const_pool.tile([P, 2 * Q], F32)
        nc.vector.memset(wd_f[:], 0.0)
        nc.vector.memset(ws_f[:], 0.0)
        mask = const_pool.tile([P, 2 * Q], F32)
        for t in range(4):
            for par in (0, 1):
                nc.gpsimd.memset(mask[:], 0.0)
                # zero-line: n - 2c + 2t - par == 0
                nc.gpsimd.affine_select(
                    out=mask[:], in_=mask[:],
                    compare_op=mybir.AluOpType.not_equal, fill=1.0,
                    base=2 * t - par, pattern=[[1, 2 * Q]],
                    channel_multiplier=-2)
                ti = 2 * t + 1 - par
                nc.vector.scalar_tensor_tensor(
                    wd_f[:], mask[:], taps_b[:, ti:ti + 1], wd_f[:],
                    op0=mybir.AluOpType.mult, op1=mybir.AluOpType.add)
                nc.vector.scalar_tensor_tensor(
                    ws_f[:], mask[:], taps_b[:, TAPS + ti:TAPS + ti + 1], ws_f[:],
                    op0=mybir.AluOpType.mult, op1=mybir.AluOpType.add)
        if mm_dt is F32:
            wd, ws_t = wd_f, ws_f
        else:
            wd = const_pool.tile([P, 2 * Q], mm_dt)
            ws_t = const_pool.tile([P, 2 * Q], mm_dt)
            nc.vector.tensor_copy(wd[:], wd_f[:])
            nc.vector.tensor_copy(ws_t[:], ws_f[:])

        def copy_act(dst, srcp):
            return nc.scalar.copy(dst, srcp)

        def copy_dve(dst, srcp):
            return nc.vector.tensor_copy(dst, srcp)

        copy_engines = [copy_act, copy_dve]

        CHUNKS = [(0, 4), (4, 8), (8, 12), (12, 17)]  # quad ranges
        def chunk_span(q0, q1):
            lo = Q * 4 * q0
            hi = min(Q * (4 * q1 - 1) + P, ME)
            return lo, hi

        for b in range(n_blocks):
            rows = slice(b * P, (b + 1) * P)

            dT = tr_pool.tile([P, NW * P], mm_dt, tag="dT")
            aT = tr_pool.tile([P, NW * P], mm_dt, tag="aT")

            stage = None
            stage_idx = 0
            for (cq0, cq1) in CHUNKS:
                lo, hi = chunk_span(cq0, cq1)
                da_t = in_pool.tile([P, 2, hi - lo], in_dt)
                src = da_ext[rows, :, lo:hi]
                if in_dt is BF16:
                    nc.gpsimd.dma_start(da_t[:], src)  # SWDGE cast f32->bf16
                else:
                    nc.sync.dma_start(da_t[:], src)

                for qd in range(cq0, cq1):
                    wins = [j for j in range(4 * qd, min(4 * qd + 4, NW))]
                    for src_i, dst in ((0, dT), (1, aT)):
                        ps = ps_t_pool.tile([P, 4 * P], in_dt)
                        for k, j in enumerate(wins):
                            _, npairs, K = win_geom(j)
                            nc.tensor.matmul(
                                ps[0:K, k * P:(k + 1) * P],
                                da_t[:, src_i, Q * j - lo: Q * j - lo + K],
                                ident[:], is_transpose=True,
                                start=(k == 0), stop=(k == len(wins) - 1))
                        eng = copy_engines[(qd + src_i) % 2]
                        eng(dst[:, 4 * qd * P: (4 * qd + len(wins)) * P],
                            ps[:, 0: len(wins) * P])
                    for g in (2 * qd, 2 * qd + 1):
                        if g >= NG:
                            continue
                        ps = ps_o_pool.tile([P, 4 * Q], F32)
                        first = True
                        off = 0
                        for j in (2 * g, 2 * g + 1):
                            _, npairs, K = win_geom(j)
                            N = 2 * npairs
                            for srcT, wmat in ((dT, wd), (aT, ws_t)):
                                nc.tensor.matmul(
                                    ps[:, off:off + N],
                                    srcT[0:K, j * P:(j + 1) * P],
                                    wmat[0:K, 0:N],
                                    start=first,
                                    stop=(j == 2 * g + 1 and srcT is aT))
                                first = False
                            off += N
                        gw = off   # 500, or 384 for the last group
                        if stage is None:
                            stage = out_pool.tile([P, 8 * 500], F32)
                            stage_off = 0
                            stage_out0 = 500 * g
                        eng = copy_engines[stage_idx % 2]
                        eng(stage[:, stage_off:stage_off + gw], ps[:, 0:gw])
                        stage_off += gw
                        if stage_off >= 8 * 500 or g == NG - 1:
                            dmae = (nc.sync, nc.scalar)[stage_idx % 2]
                            dmae.dma_start(
                                out_ext[rows,
                                        stage_out0: stage_out0 + stage_off],
                                stage[:, 0:stage_off])
                            stage = None
                            stage_idx += 1
    nc.finalize()
    return nc


def _build_v6():
    """TensorEngine kernel with overlapped transpose windows (no halo mms).

    f32r window variant: data is loaded/transposed as float32r (1.5 cyc/row
    transposes via bitcast loads), and conv matmuls pad N from 250 to 256
    (zero W columns) to hit float32r's full-rate threshold; the zero gap is
    skipped during the PSUM->SBUF staging copy via a 3D access pattern.
    """
    nc = bacc.Bacc()
    da_ext = nc.declare_dram_parameter("da", [R, 2, ME], F32, isOutput=False)
    ws_ext = nc.declare_dram_parameter("ws", [2 * TAPS], F32, isOutput=False)
    out_ext = nc.declare_dram_parameter("out", [R, 2 * M], F32, isOutput=True)

    F32R = mybir.dt.float32r
    in_dt = F32R
    mm_dt = F32R
    n_blocks = R // P          # 2 row-blocks
    Q = 125                    # pairs per window
    NW = -(-M // Q)            # 66 windows per row-block
    NQUAD = -(-NW // 4)        # 17 transpose quads
    NG = NW // 2               # 33 conv groups (2 windows each)
    inv_scale = 1.0 / 128.0
    bufs_big = 1   # tr pool (dT/aT)

    def win_geom(j):
        q0 = Q * j
        npairs = min(Q, M - q0)
        return q0, npairs, npairs + 3   # out-pair offset, n pairs, K

    with tile.TileContext(nc) as tc, ExitStack() as ctx:
        const_pool = ctx.enter_context(tc.tile_pool(name="const", bufs=1))
        in_pool = ctx.enter_context(tc.tile_pool(name="inp", bufs=3))
        tr_pool = ctx.enter_context(tc.tile_pool(name="tr", bufs=bufs_big))
        out_pool = ctx.enter_context(tc.tile_pool(name="outp", bufs=4))
        ps_t_pool = ctx.enter_context(tc.tile_pool(name="pst", bufs=3, space="PSUM"))
        ps_o_pool = ctx.enter_context(tc.tile_pool(name="pso", bufs=3, space="PSUM"))
        ps_c_pool = ctx.enter_context(tc.tile_pool(name="psc", bufs=1, space="PSUM"))

        # ---- taps broadcast (scaled by 1/128) ----
        wv_stage = const_pool.tile([1, 2 * TAPS], F32)
        nc.sync.dma_start(wv_stage[0:1, :],
                          ws_ext[:].rearrange("(o t) -> o t", o=1))
        wv = const_pool.tile([1, 2 * TAPS], F32)
        nc.vector.tensor_copy(wv[:], wv_stage[:])
        ones = const_pool.tile([1, P], F32)
        nc.vector.memset(ones[:], inv_scale)
        taps_ps = ps_c_pool.tile([P, 2 * TAPS], F32)
        nc.tensor.matmul(taps_ps[:], ones[:], wv[:], start=True, stop=True)
        taps_b = const_pool.tile([P, 2 * TAPS], F32)
        nc.vector.tensor_copy(taps_b[:], taps_ps[:])

        # ---- identity for PE transposes ----
        ident_f = const_pool.tile([P, P], F32)
        nc.gpsimd.memset(ident_f[:], 0.0)
        nc.gpsimd.affine_select(
            out=ident_f[:], in_=ident_f[:],
            compare_op=mybir.AluOpType.not_equal, fill=1.0,
            base=0, pattern=[[-1, P]], channel_multiplier=1)
        ident = const_pool.tile([P, P], in_dt)
        nc.vector.tensor_copy(ident[:], ident_f[:])

        # ---- banded weights: W[c, 2q+par] = tap[2(c-q)+1-par], 0<=c-q<4 ----
        NPAD = 256
        wd_f = const_pool.tile([P, NPAD], F32)
        ws_f = const_pool.tile([P, NPAD], F32)
        nc.vector.memset(wd_f[:], 0.0)
        nc.vector.memset(ws_f[:], 0.0)
        mask = const_pool.tile([P, 2 * Q], F32)
        for t in range(4):
            for par in (0, 1):
                nc.gpsimd.memset(mask[:], 0.0)
                # zero-line: n - 2c + 2t - par == 0
                nc.gpsimd.affine_select(
                    out=mask[:], in_=mask[:],
                    compare_op=mybir.AluOpType.not_equal, fill=1.0,
                    base=2 * t - par, pattern=[[1, 2 * Q]],
                    channel_multiplier=-2)
                ti = 2 * t + 1 - par
                nc.vector.scalar_tensor_tensor(
                    wd_f[:, 0:2 * Q], mask[:], taps_b[:, ti:ti + 1],
                    wd_f[:, 0:2 * Q],
                    op0=mybir.AluOpType.mult, op1=mybir.AluOpType.add)
                nc.vector.scalar_tensor_tensor(
                    ws_f[:, 0:2 * Q], mask[:], taps_b[:, TAPS + ti:TAPS + ti + 1],
                    ws_f[:, 0:2 * Q],
                    op0=mybir.AluOpType.mult, op1=mybir.AluOpType.add)
        wd = const_pool.tile([P, NPAD], mm_dt)
        ws_t = const_pool.tile([P, NPAD], mm_dt)
        nc.vector.tensor_copy(wd[:], wd_f[:])
        nc.vector.tensor_copy(ws_t[:], ws_f[:])

        def copy_act(dst, srcp):
            return nc.scalar.copy(dst, srcp)

        def copy_dve(dst, srcp):
            return nc.vector.tensor_copy(dst, srcp)

        copy_engines = [copy_act, copy_dve]

        CHUNKS = [(0, 4), (4, 8), (8, 12), (12, 17)]  # quad ranges
        def chunk_span(q0, q1):
            lo = Q * 4 * q0
            hi = min(Q * (4 * q1 - 1) + P, ME)
            return lo, hi

        for b in range(n_blocks):
            rows = slice(b * P, (b + 1) * P)

            dT = tr_pool.tile([P, NW * P], mm_dt, tag="dT")
            aT = tr_pool.tile([P, NW * P], mm_dt, tag="aT")

            stage = None
            stage_idx = 0
            for (cq0, cq1) in CHUNKS:
                lo, hi = chunk_span(cq0, cq1)
                da_t = in_pool.tile([P, 2, hi - lo], in_dt)
                nc.sync.dma_start(
                    da_t[:], da_ext[rows, :, lo:hi].bitcast(in_dt))

                for qd in range(cq0, cq1):
                    wins = [j for j in range(4 * qd, min(4 * qd + 4, NW))]
                    for src_i, dst in ((0, dT), (1, aT)):
                        ps = ps_t_pool.tile([P, 4 * P], in_dt)
                        for k, j in enumerate(wins):
                            _, npairs, K = win_geom(j)
                            nc.tensor.matmul(
                                ps[0:K, k * P:(k + 1) * P],
                                da_t[:, src_i, Q * j - lo: Q * j - lo + K],
                                ident[:], is_transpose=True,
                                start=(k == 0), stop=(k == len(wins) - 1))
                        eng = copy_engines[(qd + src_i) % 2]
                        eng(dst[:, 4 * qd * P: (4 * qd + len(wins)) * P],
                            ps[:, 0: len(wins) * P])
                    for g in (2 * qd, 2 * qd + 1):
                        if g >= NG:
                            continue
                        ps = ps_o_pool.tile([P, 2 * NPAD], F32)
                        first = True
                        widths = []
                        for wi, j in enumerate((2 * g, 2 * g + 1)):
                            _, npairs, K = win_geom(j)
                            N = NPAD if npairs == Q else 2 * npairs
                            for srcT, wmat in ((dT, wd), (aT, ws_t)):
                                nc.tensor.matmul(
                                    ps[:, wi * NPAD: wi * NPAD + N],
                                    srcT[0:K, j * P:(j + 1) * P],
                                    wmat[0:K, 0:N],
                                    start=first,
                                    stop=(j == 2 * g + 1 and srcT is aT))
                                first = False
                            widths.append(2 * npairs)
                        gw = sum(widths)   # 500, or 384 for the last group
                        if stage is None:
                            stage = out_pool.tile([P, 8 * 500], F32)
                            stage_off = 0
                            stage_out0 = 500 * g
                        eng = copy_engines[stage_idx % 2]
                        if widths == [2 * Q, 2 * Q]:
                            s3 = ps[:].rearrange(
                                "p (w x) -> p w x", w=2)[:, :, 0:2 * Q]
                            d3 = stage[:, stage_off:stage_off + gw].rearrange(
                                "p (w x) -> p w x", w=2)
                            eng(d3, s3)
                        else:
                            eng(stage[:, stage_off:stage_off + widths[0]],
                                ps[:, 0:widths[0]])
                            eng(stage[:, stage_off + widths[0]:
                                      stage_off + gw],
                                ps[:, NPAD:NPAD + widths[1]])
                        stage_off += gw
                        if stage_off >= 8 * 500 or g == NG - 1:
                            dmae = (nc.sync, nc.scalar)[stage_idx % 2]
                            dmae.dma_start(
                                out_ext[rows,
                                        stage_out0: stage_out0 + stage_off],
                                stage[:, 0:stage_off])
                            stage = None
                            stage_idx += 1
    nc.finalize()
    return nc


def _build_v5(mm_dtype="bf16"):
    """TensorEngine kernel with overlapped transpose windows (no halo mms).

    As v4, plus: loads are f32 on the HWDGE sync ring (full HBM rate; the
    bf16 cast rides the mandatory PSUM->SBUF transpose copy), stores go on
    the scalar HWDGE ring so load triggers are never queued behind them.
    """
    nc = bacc.Bacc()
    da_ext = nc.declare_dram_parameter("da", [R, 2, ME], F32, isOutput=False)
    ws_ext = nc.declare_dram_parameter("ws", [2 * TAPS], F32, isOutput=False)
    out_ext = nc.declare_dram_parameter("out", [R, 2 * M], F32, isOutput=True)

    BF16 = mybir.dt.bfloat16
    in_dt = F32   # loads + transposes stay f32; cast happens in PSUM->SBUF copy
    mm_dt = {"bf16": BF16, "f32r": mybir.dt.float32r, "f32": F32}[mm_dtype]
    n_blocks = R // P          # 2 row-blocks
    Q = 125                    # pairs per window
    NW = -(-M // Q)            # 66 windows per row-block
    NQUAD = -(-NW // 4)        # 17 transpose quads
    NG = NW // 2               # 33 conv groups (2 windows each)
    inv_scale = 1.0 / 128.0
    bufs_big = 2 if in_dt is BF16 else 1

    def win_geom(j):
        q0 = Q * j
        npairs = min(Q, M - q0)
        return q0, npairs, npairs + 3   # out-pair offset, n pairs, K

    with tile.TileContext(nc) as tc, ExitStack() as ctx:
        const_pool = ctx.enter_context(tc.tile_pool(name="const", bufs=1))
        in_pool = ctx.enter_context(tc.tile_pool(name="inp", bufs=3))
        tr_pool = ctx.enter_context(tc.tile_pool(name="tr", bufs=bufs_big))
        out_pool = ctx.enter_context(tc.tile_pool(name="outp", bufs=3))
        ps_t_pool = ctx.enter_context(tc.tile_pool(name="pst", bufs=3, space="PSUM"))
        ps_o_pool = ctx.enter_context(tc.tile_pool(name="pso", bufs=4, space="PSUM"))
        ps_c_pool = ctx.enter_context(tc.tile_pool(name="psc", bufs=1, space="PSUM"))

        # ---- taps broadcast (scaled by 1/128) ----
        wv_stage = const_pool.tile([1, 2 * TAPS], F32)
        nc.sync.dma_start(wv_stage[0:1, :],
                          ws_ext[:].rearrange("(o t) -> o t", o=1))
        wv = const_pool.tile([1, 2 * TAPS], F32)
        nc.vector.tensor_copy(wv[:], wv_stage[:])
        ones = const_pool.tile([1, P], F32)
        nc.vector.memset(ones[:], inv_scale)
        taps_ps = ps_c_pool.tile([P, 2 * TAPS], F32)
        nc.tensor.matmul(taps_ps[:], ones[:], wv[:], start=True, stop=True)
        taps_b = const_pool.tile([P, 2 * TAPS], F32)
        nc.vector.tensor_copy(taps_b[:], taps_ps[:])

        # ---- identity for PE transposes ----
        ident_f = const_pool.tile([P, P], F32)
        nc.gpsimd.memset(ident_f[:], 0.0)
        nc.gpsimd.affine_select(
            out=ident_f[:], in_=ident_f[:],
            compare_op=mybir.AluOpType.not_equal, fill=1.0,
            base=0, pattern=[[-1, P]], channel_multiplier=1)
        if in_dt is F32:
            ident = ident_f
        else:
            ident = const_pool.tile([P, P], in_dt)
            nc.vector.tensor_copy(ident[:], ident_f[:])

        # ---- banded weights: W[c, 2q+par] = tap[2(c-q)+1-par], 0<=c-q<4 ----
        NPAD = 256
        wd_f = const_pool.tile([P, NPAD], F32)
        ws_f = const_pool.tile([P, NPAD], F32)
        nc.vector.memset(wd_f[:], 0.0)
        nc.vector.memset(ws_f[:], 0.0)
        mask = const_pool.tile([P, 2 * Q], F32)
        for t in range(4):
            for par in (0, 1):
                nc.gpsimd.memset(mask[:], 0.0)
                # zero-line: n - 2c + 2t - par == 0
                nc.gpsimd.affine_select(
                    out=mask[:], in_=mask[:],
                    compare_op=mybir.AluOpType.not_equal, fill=1.0,
                    base=2 * t - par, pattern=[[1, 2 * Q]],
                    channel_multiplier=-2)
                ti = 2 * t + 1 - par
                nc.vector.scalar_tensor_tensor(
                    wd_f[:, 0:2 * Q], mask[:], taps_b[:, ti:ti + 1],
                    wd_f[:, 0:2 * Q],
                    op0=mybir.AluOpType.mult, op1=mybir.AluOpType.add)
                nc.vector.scalar_tensor_tensor(
                    ws_f[:, 0:2 * Q], mask[:], taps_b[:, TAPS + ti:TAPS + ti + 1],
                    ws_f[:, 0:2 * Q],
                    op0=mybir.AluOpType.mult, op1=mybir.AluOpType.add)
        if mm_dt is F32:
            wd, ws_t = wd_f, ws_f
        else:
            wd = const_pool.tile([P, 2 * Q], mm_dt)
            ws_t = const_pool.tile([P, 2 * Q], mm_dt)
            nc.vector.tensor_copy(wd[:], wd_f[:])
            nc.vector.tensor_copy(ws_t[:], ws_f[:])

        def copy_act(dst, srcp):
            return nc.scalar.copy(dst, srcp)

        def copy_dve(dst, srcp):
            return nc.vector.tensor_copy(dst, srcp)

        copy_engines = [copy_act, copy_dve]

        CHUNKS = [(0, 4), (4, 8), (8, 12), (12, 17)]  # quad ranges
        def chunk_span(q0, q1):
            lo = Q * 4 * q0
            hi = min(Q * (4 * q1 - 1) + P, ME)
            return lo, hi

        for b in range(n_blocks):
            rows = slice(b * P, (b + 1) * P)

            dT = tr_pool.tile([P, NW * P], mm_dt, tag="dT")
            aT = tr_pool.tile([P, NW * P], mm_dt, tag="aT")

            stage = None
            stage_idx = 0
            for (cq0, cq1) in CHUNKS:
                lo, hi = chunk_span(cq0, cq1)
                da_t = in_pool.tile([P, 2, hi - lo], in_dt)
                nc.sync.dma_start(da_t[:], da_ext[rows, :, lo:hi])

                for qd in range(cq0, cq1):
                    wins = [j for j in range(4 * qd, min(4 * qd + 4, NW))]
                    for src_i, dst in ((0, dT), (1, aT)):
                        ps = ps_t_pool.tile([P, 4 * P], in_dt)
                        for k, j in enumerate(wins):
                            _, npairs, K = win_geom(j)
                            nc.tensor.matmul(
                                ps[0:K, k * P:(k + 1) * P],
                                da_t[:, src_i, Q * j - lo: Q * j - lo + K],
                                ident[:], is_transpose=True,
                                start=(k == 0), stop=(k == len(wins) - 1))
                        eng = copy_engines[(qd + src_i) % 2]
                        eng(dst[:, 4 * qd * P: (4 * qd + len(wins)) * P],
                            ps[:, 0: len(wins) * P])
                    for g in (2 * qd, 2 * qd + 1):
                        if g >= NG:
                            continue
                        ps = ps_o_pool.tile([P, 4 * Q], F32)
                        first = True
                        off = 0
                        for j in (2 * g, 2 * g + 1):
                            _, npairs, K = win_geom(j)
                            N = 2 * npairs
                            for srcT, wmat in ((dT, wd), (aT, ws_t)):
                                nc.tensor.matmul(
                                    ps[:, off:off + N],
                                    srcT[0:K, j * P:(j + 1) * P],
                                    wmat[0:K, 0:N],
                                    start=first,
                                    stop=(j == 2 * g + 1 and srcT is aT))
                                first = False
                            off += N
                        gw = off   # 500, or 384 for the last group
                        if stage is None:
                            stage = out_pool.tile([P, 8 * 500], F32)
                            stage_off = 0
                            stage_out0 = 500 * g
                        eng = copy_engines[stage_idx % 2]
                        eng(stage[:, stage_off:stage_off + gw], ps[:, 0:gw])
                        stage_off += gw
                        if stage_off >= 8 * 500 or g == NG - 1:
                            nc.scalar.dma_start(
                                out_ext[rows,
                                        stage_out0: stage_out0 + stage_off],
                                stage[:, 0:stage_off])
                            stage = None
                            stage_idx += 1
    nc.finalize()
    return nc


DEFAULT_VERSION = "v6f32r"


def _get_nc(version=None):
    version = version or DEFAULT_VERSION
    if version not in _NC_CACHE:
        builders = {
            "v1": _build_v1,
            "v2bf16": lambda: _build_v2("bf16"),
            "v2f32r": lambda: _build_v2("f32r"),
            "v2f32": lambda: _build_v2("f32"),
            "v3bf16": lambda: _build_v3("bf16"),
            "v4bf16": lambda: _build_v4("bf16"),
            "v5bf16": lambda: _build_v5("bf16"),
            "v6f32r": _build_v6,
            "v5f32r": lambda: _build_v5("f32r"),
            "v5f32": lambda: _build_v5("f32"),
            "v4f32r": lambda: _build_v4("f32r"),
            "v4f32": lambda: _build_v4("f32"),
            "v3f32r": lambda: _build_v3("f32r"),
            "v3f32": lambda: _build_v3("f32"),
        }
        _NC_CACHE[version] = builders[version]()
    return _NC_CACHE[version]


def _prep_host_inputs(inputs):
    d = np.ascontiguousarray(inputs["details"], dtype=np.float32)
    a = np.ascontiguousarray(inputs["approximation"], dtype=np.float32)
    w = np.ascontiguousarray(inputs["wavelet"], dtype=np.float32)
    s = np.ascontiguousarray(inputs["scaling"], dtype=np.float32)
    da = np.empty((N_ROWS, 2, ME), dtype=np.float32)
    da[:, 0, HALO:] = d
    da[:, 0, :HALO] = d[:, M - HALO:]
    da[:, 1, HALO:] = a
    da[:, 1, :HALO] = a[:, M - HALO:]
    ws = np.concatenate([w, s])
    return da, ws


def _run(inputs, trace=False, version=None):
    nc = _get_nc(version)
    da, ws = _prep_host_inputs(inputs)
    in_maps = [
        {"da": da[i * R:(i + 1) * R], "ws": ws}
        for i in range(N_CORES)
    ]
    res = run_bass_kernel_spmd(nc, in_maps, core_ids=list(range(N_CORES)),
                               trace=trace)
    out = np.concatenate([res.results[i]["out"] for i in range(N_CORES)], axis=0)
    return out, res


def kernel(**inputs) -> np.ndarray:
    out, _ = _run(inputs, trace=False)
    return out
